# revision 1
# baseline (speedup 1.0000x reference)
"""MoE MLP (2 experts, top-1 routing) Trainium2 kernel.

Dispatch: tokens are sorted by routed expert and packed into 8
single-expert chunks.  The core split (c0 cores for expert 0, c1 = 8-c0
for expert 1) and the per-core token capacity T are chosen at runtime to
minimize T = max(ceil(n0/c0), ceil(n1/c1)) -- the per-core tensor work.
Top-1 routing sends each token to exactly one expert, so no cross-core
combine is needed; the host scatters rows back by token index.

Routing-weight folding: s(n) = top-prob of token n.  leaky_relu is
positively homogeneous and is squared, so
  s * square(leaky(x@W1.T)) @ W2.T == square(leaky((sqrt(s)*x)@W1.T)) @ W2.T
and sqrt(s) is folded into x on the host.

Device program (per core, operands host-packed, bf16 compute):
  hT = wfc @ xs        ([H,T], PSUM, 128x128 weight tiles, contraction D)
  aT = sq(lrelu(hT, 0.5))            (bf16, held in SBUF)
  yT = wpj @ aT        ([D,T], contraction H)  -> fp32 out

DMA layouts (per-partition line size drives per-DMA-engine throughput --
measured ~13GB/s/engine at 520B lines vs ~25GB/s at 2KB, x16 engines):
  xsT[k]  [4, P, 4, tb_k]       4 lines of ~2KB per partition
  wfcT    [P, NPAN1, KB1, 256]  fc weights, 8KB/partition per panel
  wpjT    [P, NPAN2, KB2, 256]  proj weights, 32KB/partition per panel
  yT      [P, KB1, T] fp32
Schedule: ~16 junk warmup matmuls un-gate the PE clock (HAM) while the
first DMAs land; wfc panel 0 arrives in 4 kb-chunks so the first matmul
group starts as soon as xs block 0 + 256KB of weights are in; wpj panels
0-1 prefetch in 1MB quarters interleaved with phase-1 wfc loads so the
phase-1 -> phase-2 transition never stalls on DMA.  Measured (8 cores):
478us cool, ~571us when the chip's P0 power state caps the PE at 2GHz;
PE issue gap is at the warm-clock floor (111ns for 260-col streams).
"""

from collections import deque

import numpy as np
import ml_dtypes

P = 128
DIM = 2048
HID = 8192
NEXP = 2
NCORES = 8
NTOK = 4096
KB1 = DIM // P           # 16  fc contraction blocks
KB2 = HID // P           # 64  proj contraction blocks
HPAN = 2                 # h-blocks per fc weight panel
DPAN = 2                 # d-blocks per proj weight panel
NPAN1 = KB2 // HPAN      # 32
NPAN2 = KB1 // DPAN      # 8

_NC_CACHE = {}
_RUN_CACHE = {}
_W_CACHE = {}


# --------------------------------------------------------------------------
# device program
# --------------------------------------------------------------------------
def _build_nc(T, tbs):
    import concourse.mybir as mybir
    import concourse.tile as tile
    from concourse import bacc

    dt = mybir.dt
    nc = bacc.Bacc(None, target_bir_lowering=False)
    # chunk-major layout: per partition, 4 DMA lines of ~2KB.  Line size
    # drives per-DMA-engine throughput (measured 13GB/s at 520B lines vs
    # 25GB/s at 2KB, x16 engines), and xs gates the first matmul group.
    xsT = [nc.dram_tensor(f"xsT{i}", [4, P, 4, tb], dt.bfloat16,
                          kind="ExternalInput").rearrange(
                              "c p k t -> p c k t")
           for i, tb in enumerate(tbs)]
    wfcT = nc.dram_tensor("wfcT", [P, NPAN1, KB1, HPAN * P], dt.bfloat16,
                          kind="ExternalInput")
    wpjT = nc.dram_tensor("wpjT", [P, NPAN2, KB2, DPAN * P], dt.bfloat16,
                          kind="ExternalInput")
    yT = nc.dram_tensor("yT", [P, KB1, T], dt.float32, kind="ExternalOutput")

    assert sum(tbs) == T and all(tb <= 512 for tb in tbs)
    toff = [sum(tbs[:i]) for i in range(len(tbs))]
    # phase-1 panel index -> list of (wpj_panel, quarter) prefetches
    wpj_pre = {}
    npre = min(2, NPAN2)
    for i in range(npre * 4):
        wpj_pre.setdefault(8 + 2 * i, []).append((i // 4, i % 4))
    qkb = KB2 // 4           # kb-blocks per prefetch quarter

    with tile.TileContext(nc) as tc:
        with tc.tile_pool(name="xs", bufs=1) as xs_pool, \
             tc.tile_pool(name="wfc", bufs=3) as wfc_pool, \
             tc.tile_pool(name="wpj", bufs=2) as wpj_pool, \
             tc.tile_pool(name="a", bufs=1) as a_pool, \
             tc.tile_pool(name="g", bufs=3) as g_pool, \
             tc.tile_pool(name="ps", bufs=8, space="PSUM") as ps_pool, \
             tc.tile_pool(name="ot", bufs=3) as out_pool:

            def load_wfc(pan, chunks=1):
                t = wfc_pool.tile([P, KB1, HPAN * P], dt.bfloat16,
                                  name="wfc_sb", tag="wfc")
                ck = KB1 // chunks
                for c in range(chunks):
                    nc.sync.dma_start(t[:, c * ck:(c + 1) * ck, :],
                                      wfcT[:, pan, c * ck:(c + 1) * ck, :])
                return t

            # HAM warmup: junk matmuls keep the PE busy while the first
            # DMAs land, so real matmuls start at the 2.4GHz clock and
            # the HAM never sees an idle window before they begin.
            wu = xs_pool.tile([P, P + tbs[0]], dt.bfloat16,
                              name="wu", tag="wu")
            nc.vector.memset(wu, 0.0)
            ps_w = ps_pool.tile([P, tbs[0]], dt.float32, tag="ps")
            for _ in range(24):
                nc.tensor.matmul(ps_w, wu[:, :P], wu[:, P:],
                                 start=True, stop=True)

            # startup order: xs block 0 first, then wfc panel 0 in
            # kb-chunks -- the first matmul group only needs xs0 plus the
            # first chunk, so it starts ~5us earlier than whole-panel DMA
            xs_sb = []
            for i, tb in enumerate(tbs):
                # distinct tags: both token blocks stay live all of phase 1
                t = xs_pool.tile([P, 4, 4, tb], dt.bfloat16,
                                 name=f"xs{i}", tag=f"xs{i}")
                nc.sync.dma_start(t, xsT[i])
                xs_sb.append(t)
                if i == 0:
                    wfc_q = deque([load_wfc(0, chunks=4)])
            for pan in range(1, min(3, NPAN1)):
                wfc_q.append(load_wfc(pan))

            aT = a_pool.tile([P, KB2, T], dt.bfloat16)
            wpj_tiles = {}

            # ---- phase 1: hT = wfc @ xs; aT = sq(lrelu(hT, 0.5)) ----
            for pan in range(NPAN1):
                wfc_sb = wfc_q.popleft()
                if pan + 3 < NPAN1:
                    wfc_q.append(load_wfc(pan + 3))
                for wp, q in wpj_pre.get(pan, []):
                    if wp not in wpj_tiles:
                        wpj_tiles[wp] = wpj_pool.tile(
                            [P, KB2, DPAN * P], dt.bfloat16,
                            name=f"wpj_sb{wp}", tag="wpj")
                    nc.sync.dma_start(
                        wpj_tiles[wp][:, q * qkb:(q + 1) * qkb, :],
                        wpjT[:, wp, q * qkb:(q + 1) * qkb, :])
                # panel 0: ti-outer so the first groups only need xs
                # block 0 (xs1's DMA is still in flight at that point)
                if pan == 0:
                    groups = [(hb, ti) for ti in range(len(tbs))
                              for hb in range(HPAN)]
                else:
                    groups = [(hb, ti) for hb in range(HPAN)
                              for ti in range(len(tbs))]
                for hb, ti in groups:
                    tb = tbs[ti]
                    t0 = toff[ti]
                    ps = ps_pool.tile([P, tb], dt.float32, tag="ps")
                    for kb in range(KB1):
                        nc.tensor.matmul(
                            ps,
                            wfc_sb[:, kb, hb * P:(hb + 1) * P],
                            xs_sb[ti][:, kb // 4, kb % 4, :],
                            start=(kb == 0), stop=(kb == KB1 - 1))
                    # sq(lrelu(h,.5)) == Square(0.5*(h + relu(h)))
                    # (ActivationFunctionType.Lrelu ignores alpha on HW)
                    r = g_pool.tile([P, tb], dt.float32, tag="r")
                    nc.scalar.activation(
                        r, ps, mybir.ActivationFunctionType.Relu)
                    s = g_pool.tile([P, tb], dt.float32, tag="s")
                    nc.vector.tensor_add(out=s, in0=ps, in1=r)
                    nc.scalar.activation(
                        aT[:, pan * HPAN + hb, t0:t0 + tb],
                        s, mybir.ActivationFunctionType.Square,
                        scale=0.5)

            # ---- phase 2: yT = wpj @ aT ----
            for pan in range(NPAN2):
                if pan in wpj_tiles:
                    wpj_sb = wpj_tiles.pop(pan)
                else:
                    wpj_sb = wpj_pool.tile([P, KB2, DPAN * P], dt.bfloat16,
                                           tag="wpj")
                    nc.sync.dma_start(wpj_sb, wpjT[:, pan])
                for db in range(DPAN):
                    for ti, tb in enumerate(tbs):
                        t0 = toff[ti]
                        ps = ps_pool.tile([P, tb], dt.float32, tag="ps")
                        for kb in range(KB2):
                            nc.tensor.matmul(
                                ps,
                                wpj_sb[:, kb, db * P:(db + 1) * P],
                                aT[:, kb, t0:t0 + tb],
                                start=(kb == 0), stop=(kb == KB2 - 1))
                        ot = out_pool.tile([P, tb], dt.float32, tag="o")
                        nc.vector.tensor_copy(ot, ps)
                        nc.sync.dma_start(
                            yT[:, pan * DPAN + db, t0:t0 + tb], ot)
    nc.compile()
    return nc


def get_nc(T, tbs):
    key = (T, tbs)
    if key not in _NC_CACHE:
        _NC_CACHE[key] = _build_nc(T, tbs)
    return _NC_CACHE[key]


# --------------------------------------------------------------------------
# runner: build the sharded jit once per nc, reuse across calls
# --------------------------------------------------------------------------
def get_runner(nc, n_cores=NCORES):
    """Returns (fn, in_names, out_names, out_shapes).  fn takes
    [n_cores*dim0, ...] concatenated inputs + zero output buffers and
    returns concatenated outputs (mirrors bass2jax.run_bass_via_pjrt,
    but the jitted callable is cached so repeat calls don't recompile)."""
    key = id(nc)
    if key in _RUN_CACHE:
        return _RUN_CACHE[key]

    import jax
    import concourse.mybir as mybir
    from concourse.bass2jax import (
        _bass_exec_p, install_neuronx_cc_hook, partition_id_tensor)
    from jax.sharding import Mesh, PartitionSpec
    try:
        from jax.experimental.shard_map import shard_map
    except ImportError:
        from jax.shard_map import shard_map

    install_neuronx_cc_hook()

    part_name = (nc.partition_id_tensor.name
                 if nc.partition_id_tensor else None)
    in_names, out_names, out_avals = [], [], []
    for alloc in nc.m.functions[0].allocations:
        if not isinstance(alloc, mybir.MemoryLocationSet):
            continue
        name = alloc.memorylocations[0].name
        if alloc.kind == "ExternalInput":
            if name != part_name:
                in_names.append(name)
        elif alloc.kind == "ExternalOutput":
            out_names.append(name)
            out_avals.append(jax.core.ShapedArray(
                tuple(alloc.tensor_shape), mybir.dt.np(alloc.dtype)))
    n_params = len(in_names)
    n_outs = len(out_names)
    all_names = in_names + out_names
    if part_name is not None:
        all_names = all_names + [part_name]
    donate = tuple(range(n_params, n_params + n_outs))

    def _body(*args):
        operands = list(args)
        if part_name is not None:
            operands.append(partition_id_tensor())
        outs = _bass_exec_p.bind(
            *operands,
            out_avals=tuple(out_avals),
            in_names=tuple(all_names),
            out_names=tuple(out_names),
            lowering_input_output_aliases=(),
            sim_require_finite=True,
            sim_require_nnan=True,
            nc=nc,
        )
        return tuple(outs)

    devices = jax.devices()[:n_cores]
    mesh = Mesh(np.asarray(devices), ("core",))
    in_specs = (PartitionSpec("core"),) * (n_params + n_outs)
    out_specs = (PartitionSpec("core"),) * n_outs
    fn = jax.jit(
        shard_map(_body, mesh=mesh, in_specs=in_specs,
                  out_specs=out_specs, check_rep=False),
        donate_argnums=donate, keep_unused=True)
    out_shapes = [(tuple(a.shape), a.dtype) for a in out_avals]
    _RUN_CACHE[key] = (fn, in_names, out_names, out_shapes)
    return _RUN_CACHE[key]


def run_spmd(nc, in_maps, n_cores=NCORES):
    fn, in_names, out_names, out_shapes = get_runner(nc, n_cores)
    concat_in = [np.concatenate([m[n] for m in in_maps], axis=0)
                 for n in in_names]
    zeros = [np.zeros((n_cores * sh[0], *sh[1:]), dt)
             for sh, dt in out_shapes]
    outs = fn(*concat_in, *zeros)
    res = []
    for c in range(n_cores):
        res.append({
            name: np.asarray(outs[i]).reshape(n_cores, *out_shapes[i][0])[c]
            for i, name in enumerate(out_names)})
    return res


# --------------------------------------------------------------------------
# host dispatch
# --------------------------------------------------------------------------
def _route(x, w_router):
    """fp32 router matching reference: top = argmax(logits) (tie -> 0),
    s = top softmax prob = sigmoid(l_top - l_other)."""
    x_flat = np.asarray(x, dtype=np.float32).reshape(-1, x.shape[-1])
    L = x_flat @ np.asarray(w_router, dtype=np.float32).T
    top = (L[:, 1] > L[:, 0])
    dlt = np.abs(L[:, 1] - L[:, 0]).astype(np.float32)
    ptop = 1.0 / (1.0 + np.exp(-dlt))
    return x_flat, top, np.sqrt(ptop).astype(np.float32)


def _plan(n0, n1):
    """Core split minimizing per-core capacity T (the PE time scales
    linearly with T, so no rounding: every column costs ~0.85us/512)."""
    best = None
    for c0 in range(NCORES + 1):
        c1 = NCORES - c0
        if (n0 > 0 and c0 == 0) or (n1 > 0 and c1 == 0):
            continue
        T = max(-(-n0 // c0) if c0 else 0, -(-n1 // c1) if c1 else 0, 8)
        if best is None or T < best[0]:
            best = (T, c0)
    return best


def _pack_weights(w_fc, w_proj):
    """Panel-contiguous bf16 layouts (cached across calls; the harness
    reuses the same arrays).  wfcT[p,pan,kb,j] = w_fc[pan*256+j, kb*128+p];
    wpjT[p,pan,kb,j] = w_proj[pan*256+j, kb*128+p]."""
    key = (id(w_fc), id(w_proj))
    hit = _W_CACHE.get(key)
    if hit is not None and hit[0] is w_fc and hit[1] is w_proj:
        return hit[2], hit[3]
    bf16 = ml_dtypes.bfloat16
    wfcT, wpjT = [], []
    for e in range(NEXP):
        a = np.asarray(w_fc[e], np.float32).astype(bf16)
        wfcT.append(np.ascontiguousarray(
            a.reshape(NPAN1, HPAN * P, KB1, P).transpose(3, 0, 2, 1)))
        b = np.asarray(w_proj[e], np.float32).astype(bf16)
        wpjT.append(np.ascontiguousarray(
            b.reshape(NPAN2, DPAN * P, KB2, P).transpose(3, 0, 2, 1)))
    _W_CACHE.clear()
    _W_CACHE[key] = (w_fc, w_proj, wfcT, wpjT)
    return wfcT, wpjT


def prepare(x, w_router, w_fc, w_proj):
    """Host dispatch: returns (nc, in_maps, assemble) so the same device
    program can be run via the cached jit path (kernel) or via
    run_bass_kernel_spmd with tracing (bench)."""
    bsz, seq, d = x.shape
    N = bsz * seq
    assert d == DIM and N == NTOK
    bf16 = ml_dtypes.bfloat16

    x_flat, top, sq = _route(x, w_router)
    n1 = int(top.sum())
    n0 = N - n1
    T, c0 = _plan(n0, n1)
    tbs = (T,) if T <= 512 else ((T // 2 + 3) // 4 * 4, 0)
    if len(tbs) == 2:
        tbs = (tbs[0], T - tbs[0])

    wfcT, wpjT = _pack_weights(w_fc, w_proj)

    # sort tokens by expert into single-expert chunks of capacity T
    perm0 = np.nonzero(~top)[0]
    perm1 = np.nonzero(top)[0]
    xs_all = np.zeros((NCORES * T, DIM), dtype=np.float32)
    tok_of_slot = np.full(NCORES * T, -1, dtype=np.int64)
    xs_scaled = x_flat * sq[:, None]
    xs_all[:n0] = xs_scaled[perm0]
    tok_of_slot[:n0] = perm0
    off1 = c0 * T
    xs_all[off1:off1 + n1] = xs_scaled[perm1]
    tok_of_slot[off1:off1 + n1] = perm1

    toff = [sum(tbs[:i]) for i in range(len(tbs))]
    in_maps = []
    for c in range(NCORES):
        e = 0 if c < c0 else 1
        xc = xs_all[c * T:(c + 1) * T].astype(bf16)      # [T, D]
        m = {"wfcT": wfcT[e], "wpjT": wpjT[e]}
        for i, tb in enumerate(tbs):
            blk = xc[toff[i]:toff[i] + tb]               # [tb, D]
            # [c, p, k, t] with d = (c*4+k)*128 + p
            m[f"xsT{i}"] = np.ascontiguousarray(
                blk.T.reshape(4, 4, P, tb).transpose(0, 2, 1, 3))
        in_maps.append(m)

    nc = get_nc(T, tbs)

    def assemble(res):
        out_flat = np.zeros((N, DIM), dtype=np.float32)
        for c in range(NCORES):
            toks = tok_of_slot[c * T:(c + 1) * T]
            valid = toks >= 0
            if valid.any():
                # yT [P, KB1, T] -> [T, D] with d = db*128 + p
                y = res[c]["yT"].transpose(2, 1, 0).reshape(T, DIM)
                out_flat[toks[valid]] = y[valid]
        return out_flat.reshape(bsz, seq, d)

    return nc, in_maps, assemble


def kernel(x, w_router, w_fc, w_proj):
    nc, in_maps, assemble = prepare(x, w_router, w_fc, w_proj)
    res = run_spmd(nc, in_maps)
    return assemble(res)



# revision 5
# speedup vs baseline: 15.9075x; 15.9075x over previous
"""MoE MLP (2 experts, top-1 routing) Trainium2 kernel.

Dispatch: tokens are sorted by routed expert and packed into 8
single-expert chunks.  The core split (c0 cores for expert 0, c1 = 8-c0
for expert 1) and the per-core token capacity T are chosen at runtime to
minimize T = max(ceil(n0/c0), ceil(n1/c1)) -- the per-core tensor work.
Top-1 routing sends each token to exactly one expert, so no cross-core
combine is needed; the host scatters rows back by token index.

Routing-weight folding: s(n) = top-prob of token n.  leaky_relu is
positively homogeneous and is squared, so
  s * square(leaky(x@W1.T)) @ W2.T == square(leaky((sqrt(s)*x)@W1.T)) @ W2.T
and sqrt(s) is folded into x on the host.

Device program (per core, operands host-packed, bf16 compute):
  hT = wfc @ xs        ([H,T], PSUM, 128x128 weight tiles, contraction D)
  aT = sq(lrelu(hT, 0.5))            (bf16, held in SBUF)
  yT = wpj @ aT        ([D,T], contraction H)  -> fp32 out

DMA layouts (per-partition line size drives per-DMA-engine throughput --
measured ~13GB/s/engine at 520B lines vs ~25GB/s at 2KB, x16 engines):
  xsT[k]  [4, P, 4, tb_k]       4 lines of ~2KB per partition
  wfcT    [P, NPAN1, KB1, 256]  fc weights, 8KB/partition per panel
  wpjT    [P, NPAN2, KB2, 256]  proj weights, 32KB/partition per panel
  yT      [P, KB1, T] fp32
Schedule: ~16 junk warmup matmuls un-gate the PE clock (HAM) while the
first DMAs land; wfc panel 0 arrives in 4 kb-chunks so the first matmul
group starts as soon as xs block 0 + 256KB of weights are in; wpj panels
0-1 prefetch in 1MB quarters interleaved with phase-1 wfc loads so the
phase-1 -> phase-2 transition never stalls on DMA.  Measured (8 cores):
478us cool, ~571us when the chip's P0 power state caps the PE at 2GHz;
PE issue gap is at the warm-clock floor (111ns for 260-col streams).
"""

from collections import deque

import numpy as np
import ml_dtypes

P = 128
DIM = 2048
HID = 8192
NEXP = 2
NCORES = 8
NTOK = 4096
KB1 = DIM // P           # 16  fc contraction blocks
KB2 = HID // P           # 64  proj contraction blocks
HPAN = 2                 # h-blocks per fc weight panel
DPAN = 2                 # d-blocks per proj weight panel
NPAN1 = KB2 // HPAN      # 32
NPAN2 = KB1 // DPAN      # 8

_NC_CACHE = {}
_RUN_CACHE = {}
_W_CACHE = {}


# --------------------------------------------------------------------------
# device program
# --------------------------------------------------------------------------
def _build_nc(T, tbs, reps=1):
    import concourse.mybir as mybir
    import concourse.tile as tile
    from concourse import bacc

    dt = mybir.dt
    nc = bacc.Bacc(None, target_bir_lowering=False)
    # chunk-major layout: per partition, 4 DMA lines of ~2KB.  Line size
    # drives per-DMA-engine throughput (measured 13GB/s at 520B lines vs
    # 25GB/s at 2KB, x16 engines), and xs gates the first matmul group.
    xsT = [nc.dram_tensor(f"xsT{i}", [4, P, 4, tb], dt.bfloat16,
                          kind="ExternalInput").rearrange(
                              "c p k t -> p c k t")
           for i, tb in enumerate(tbs)]
    wfcT = nc.dram_tensor("wfcT", [P, NPAN1, KB1, HPAN * P], dt.bfloat16,
                          kind="ExternalInput")
    wpjT = nc.dram_tensor("wpjT", [P, NPAN2, KB2, DPAN * P], dt.bfloat16,
                          kind="ExternalInput")
    yT = nc.dram_tensor("yT", [P, KB1, T], dt.float32, kind="ExternalOutput")

    assert sum(tbs) == T and all(tb <= 512 for tb in tbs)
    toff = [sum(tbs[:i]) for i in range(len(tbs))]
    # phase-1 panel index -> list of (wpj_panel, quarter) prefetches
    wpj_pre = {}
    npre = min(2, NPAN2)
    for i in range(npre * 4):
        wpj_pre.setdefault(8 + 2 * i, []).append((i // 4, i % 4))
    qkb = KB2 // 4           # kb-blocks per prefetch quarter

    with tile.TileContext(nc) as tc:
        with tc.tile_pool(name="xs", bufs=1) as xs_pool, \
             tc.tile_pool(name="wfc", bufs=3) as wfc_pool, \
             tc.tile_pool(name="wpj", bufs=2) as wpj_pool, \
             tc.tile_pool(name="a", bufs=1) as a_pool, \
             tc.tile_pool(name="g", bufs=3) as g_pool, \
             tc.tile_pool(name="ps", bufs=8, space="PSUM") as ps_pool, \
             tc.tile_pool(name="ot", bufs=3) as out_pool:

            def load_wfc(pan, chunks=1):
                t = wfc_pool.tile([P, KB1, HPAN * P], dt.bfloat16,
                                  name="wfc_sb", tag="wfc")
                ck = KB1 // chunks
                for c in range(chunks):
                    nc.sync.dma_start(t[:, c * ck:(c + 1) * ck, :],
                                      wfcT[:, pan, c * ck:(c + 1) * ck, :])
                return t

            # HAM warmup: junk matmuls keep the PE busy while the first
            # DMAs land, so real matmuls start at the 2.4GHz clock and
            # the HAM never sees an idle window before they begin.
            wu = xs_pool.tile([P, P + tbs[0]], dt.bfloat16,
                              name="wu", tag="wu")
            nc.vector.memset(wu, 0.0)
            ps_w = ps_pool.tile([P, tbs[0]], dt.float32, tag="ps")
            for _ in range(24):
                nc.tensor.matmul(ps_w, wu[:, :P], wu[:, P:],
                                 start=True, stop=True)

            for _rep in range(reps):
                _body(nc, tile, mybir, dt, T, tbs, toff, wpj_pre, qkb,
                      xsT, wfcT, wpjT, yT, load_wfc,
                      xs_pool, wfc_pool, wpj_pool, a_pool, g_pool,
                      ps_pool, out_pool)
    nc.compile()
    return nc


def _body(nc, tile, mybir, dt, T, tbs, toff, wpj_pre, qkb,
          xsT, wfcT, wpjT, yT, load_wfc,
          xs_pool, wfc_pool, wpj_pool, a_pool, g_pool, ps_pool, out_pool):
    # startup order: xs block 0 first, then wfc panel 0 in
    # kb-chunks -- the first matmul group only needs xs0 plus the
    # first chunk, so it starts ~5us earlier than whole-panel DMA
    xs_sb = []
    for i, tb in enumerate(tbs):
        # distinct tags: both token blocks stay live all of phase 1
        t = xs_pool.tile([P, 4, 4, tb], dt.bfloat16,
                         name=f"xs{i}", tag=f"xs{i}")
        nc.sync.dma_start(t, xsT[i])
        xs_sb.append(t)
        if i == 0:
            wfc_q = deque([load_wfc(0, chunks=4)])
    for pan in range(1, min(3, NPAN1)):
        wfc_q.append(load_wfc(pan))

    aT = a_pool.tile([P, KB2, T], dt.bfloat16, tag="aT")
    wpj_tiles = {}

    # ---- phase 1: hT = wfc @ xs; aT = sq(lrelu(hT, 0.5)) ----
    for pan in range(NPAN1):
        wfc_sb = wfc_q.popleft()
        if pan + 3 < NPAN1:
            wfc_q.append(load_wfc(pan + 3))
        for wp, q in wpj_pre.get(pan, []):
            if wp not in wpj_tiles:
                wpj_tiles[wp] = wpj_pool.tile(
                    [P, KB2, DPAN * P], dt.bfloat16,
                    name=f"wpj_sb{wp}", tag="wpj")
            nc.sync.dma_start(
                wpj_tiles[wp][:, q * qkb:(q + 1) * qkb, :],
                wpjT[:, wp, q * qkb:(q + 1) * qkb, :])
        # panel 0: ti-outer so the first groups only need xs
        # block 0 (xs1's DMA is still in flight at that point)
        if pan == 0:
            groups = [(hb, ti) for ti in range(len(tbs))
                      for hb in range(HPAN)]
        else:
            groups = [(hb, ti) for hb in range(HPAN)
                      for ti in range(len(tbs))]
        for hb, ti in groups:
            tb = tbs[ti]
            t0 = toff[ti]
            ps = ps_pool.tile([P, tb], dt.float32, tag="ps")
            for kb in range(KB1):
                nc.tensor.matmul(
                    ps,
                    wfc_sb[:, kb, hb * P:(hb + 1) * P],
                    xs_sb[ti][:, kb // 4, kb % 4, :],
                    start=(kb == 0), stop=(kb == KB1 - 1))
            # sq(lrelu(h,.5)) == Square(0.5*(h + relu(h)))
            # (ActivationFunctionType.Lrelu ignores alpha on HW)
            r = g_pool.tile([P, tb], dt.float32, tag="r")
            nc.scalar.activation(
                r, ps, mybir.ActivationFunctionType.Relu)
            s = g_pool.tile([P, tb], dt.float32, tag="s")
            nc.vector.tensor_add(out=s, in0=ps, in1=r)
            nc.scalar.activation(
                aT[:, pan * HPAN + hb, t0:t0 + tb],
                s, mybir.ActivationFunctionType.Square,
                scale=0.5)

    # ---- phase 2: yT = wpj @ aT ----
    for pan in range(NPAN2):
        if pan in wpj_tiles:
            wpj_sb = wpj_tiles.pop(pan)
        else:
            wpj_sb = wpj_pool.tile([P, KB2, DPAN * P], dt.bfloat16,
                                   tag="wpj")
            nc.sync.dma_start(wpj_sb, wpjT[:, pan])
        for db in range(DPAN):
            for ti, tb in enumerate(tbs):
                t0 = toff[ti]
                ps = ps_pool.tile([P, tb], dt.float32, tag="ps")
                for kb in range(KB2):
                    nc.tensor.matmul(
                        ps,
                        wpj_sb[:, kb, db * P:(db + 1) * P],
                        aT[:, kb, t0:t0 + tb],
                        start=(kb == 0), stop=(kb == KB2 - 1))
                ot = out_pool.tile([P, tb], dt.float32, tag="o")
                nc.vector.tensor_copy(ot, ps)
                nc.sync.dma_start(
                    yT[:, pan * DPAN + db, t0:t0 + tb], ot)


def get_nc(T, tbs, reps=1):
    key = (T, tbs, reps)
    if key not in _NC_CACHE:
        _NC_CACHE[key] = _build_nc(T, tbs, reps)
    return _NC_CACHE[key]


# --------------------------------------------------------------------------
# runner: build the sharded jit once per nc, reuse across calls
# --------------------------------------------------------------------------
def get_runner(nc, n_cores=NCORES):
    """Returns (fn, in_names, out_names, out_shapes).  fn takes
    [n_cores*dim0, ...] concatenated inputs + zero output buffers and
    returns concatenated outputs (mirrors bass2jax.run_bass_via_pjrt,
    but the jitted callable is cached so repeat calls don't recompile)."""
    key = id(nc)
    if key in _RUN_CACHE:
        return _RUN_CACHE[key]

    import jax
    import concourse.mybir as mybir
    from concourse.bass2jax import (
        _bass_exec_p, install_neuronx_cc_hook, partition_id_tensor)
    from jax.sharding import Mesh, PartitionSpec
    try:
        from jax.experimental.shard_map import shard_map
    except ImportError:
        from jax.shard_map import shard_map

    install_neuronx_cc_hook()

    part_name = (nc.partition_id_tensor.name
                 if nc.partition_id_tensor else None)
    in_names, out_names, out_avals = [], [], []
    for alloc in nc.m.functions[0].allocations:
        if not isinstance(alloc, mybir.MemoryLocationSet):
            continue
        name = alloc.memorylocations[0].name
        if alloc.kind == "ExternalInput":
            if name != part_name:
                in_names.append(name)
        elif alloc.kind == "ExternalOutput":
            out_names.append(name)
            out_avals.append(jax.core.ShapedArray(
                tuple(alloc.tensor_shape), mybir.dt.np(alloc.dtype)))
    n_params = len(in_names)
    n_outs = len(out_names)
    all_names = in_names + out_names
    if part_name is not None:
        all_names = all_names + [part_name]
    donate = tuple(range(n_params, n_params + n_outs))

    def _body(*args):
        operands = list(args)
        if part_name is not None:
            operands.append(partition_id_tensor())
        outs = _bass_exec_p.bind(
            *operands,
            out_avals=tuple(out_avals),
            in_names=tuple(all_names),
            out_names=tuple(out_names),
            lowering_input_output_aliases=(),
            sim_require_finite=True,
            sim_require_nnan=True,
            nc=nc,
        )
        return tuple(outs)

    devices = jax.devices()[:n_cores]
    mesh = Mesh(np.asarray(devices), ("core",))
    in_specs = (PartitionSpec("core"),) * (n_params + n_outs)
    out_specs = (PartitionSpec("core"),) * n_outs
    fn = jax.jit(
        shard_map(_body, mesh=mesh, in_specs=in_specs,
                  out_specs=out_specs, check_rep=False),
        donate_argnums=donate, keep_unused=True)
    out_shapes = [(tuple(a.shape), a.dtype) for a in out_avals]
    _RUN_CACHE[key] = (fn, in_names, out_names, out_shapes)
    return _RUN_CACHE[key]


def run_spmd(nc, in_maps, n_cores=NCORES):
    fn, in_names, out_names, out_shapes = get_runner(nc, n_cores)
    concat_in = [np.concatenate([m[n] for m in in_maps], axis=0)
                 for n in in_names]
    zeros = [np.zeros((n_cores * sh[0], *sh[1:]), dt)
             for sh, dt in out_shapes]
    outs = fn(*concat_in, *zeros)
    res = []
    for c in range(n_cores):
        res.append({
            name: np.asarray(outs[i]).reshape(n_cores, *out_shapes[i][0])[c]
            for i, name in enumerate(out_names)})
    return res


# --------------------------------------------------------------------------
# host dispatch
# --------------------------------------------------------------------------
def _route(x, w_router):
    """fp32 router matching reference: top = argmax(logits) (tie -> 0),
    s = top softmax prob = sigmoid(l_top - l_other)."""
    x_flat = np.asarray(x, dtype=np.float32).reshape(-1, x.shape[-1])
    L = x_flat @ np.asarray(w_router, dtype=np.float32).T
    top = (L[:, 1] > L[:, 0])
    dlt = np.abs(L[:, 1] - L[:, 0]).astype(np.float32)
    ptop = 1.0 / (1.0 + np.exp(-dlt))
    return x_flat, top, np.sqrt(ptop).astype(np.float32)


def _plan(n0, n1):
    """Core split minimizing per-core capacity T (the PE time scales
    linearly with T, so no rounding: every column costs ~0.85us/512)."""
    best = None
    for c0 in range(NCORES + 1):
        c1 = NCORES - c0
        if (n0 > 0 and c0 == 0) or (n1 > 0 and c1 == 0):
            continue
        T = max(-(-n0 // c0) if c0 else 0, -(-n1 // c1) if c1 else 0, 8)
        if best is None or T < best[0]:
            best = (T, c0)
    return best


def _pack_weights(w_fc, w_proj):
    """Panel-contiguous bf16 layouts (cached across calls; the harness
    reuses the same arrays).  wfcT[p,pan,kb,j] = w_fc[pan*256+j, kb*128+p];
    wpjT[p,pan,kb,j] = w_proj[pan*256+j, kb*128+p]."""
    key = (id(w_fc), id(w_proj))
    hit = _W_CACHE.get(key)
    if hit is not None and hit[0] is w_fc and hit[1] is w_proj:
        return hit[2], hit[3]
    bf16 = ml_dtypes.bfloat16
    wfcT, wpjT = [], []
    for e in range(NEXP):
        a = np.asarray(w_fc[e], np.float32).astype(bf16)
        wfcT.append(np.ascontiguousarray(
            a.reshape(NPAN1, HPAN * P, KB1, P).transpose(3, 0, 2, 1)))
        b = np.asarray(w_proj[e], np.float32).astype(bf16)
        wpjT.append(np.ascontiguousarray(
            b.reshape(NPAN2, DPAN * P, KB2, P).transpose(3, 0, 2, 1)))
    _W_CACHE.clear()
    _W_CACHE[key] = (w_fc, w_proj, wfcT, wpjT)
    return wfcT, wpjT


def prepare(x, w_router, w_fc, w_proj):
    """Host dispatch: returns (nc, in_maps, assemble) so the same device
    program can be run via the cached jit path (kernel) or via
    run_bass_kernel_spmd with tracing (bench)."""
    bsz, seq, d = x.shape
    N = bsz * seq
    assert d == DIM and N == NTOK
    bf16 = ml_dtypes.bfloat16

    x_flat, top, sq = _route(x, w_router)
    n1 = int(top.sum())
    n0 = N - n1
    T, c0 = _plan(n0, n1)
    tbs = (T,) if T <= 512 else ((T // 2 + 3) // 4 * 4, 0)
    if len(tbs) == 2:
        tbs = (tbs[0], T - tbs[0])
    global _LAST_PLAN
    _LAST_PLAN = (T, tbs)

    wfcT, wpjT = _pack_weights(w_fc, w_proj)

    # sort tokens by expert into single-expert chunks of capacity T
    perm0 = np.nonzero(~top)[0]
    perm1 = np.nonzero(top)[0]
    xs_all = np.zeros((NCORES * T, DIM), dtype=np.float32)
    tok_of_slot = np.full(NCORES * T, -1, dtype=np.int64)
    xs_scaled = x_flat * sq[:, None]
    xs_all[:n0] = xs_scaled[perm0]
    tok_of_slot[:n0] = perm0
    off1 = c0 * T
    xs_all[off1:off1 + n1] = xs_scaled[perm1]
    tok_of_slot[off1:off1 + n1] = perm1

    toff = [sum(tbs[:i]) for i in range(len(tbs))]
    in_maps = []
    for c in range(NCORES):
        e = 0 if c < c0 else 1
        xc = xs_all[c * T:(c + 1) * T].astype(bf16)      # [T, D]
        m = {"wfcT": wfcT[e], "wpjT": wpjT[e]}
        for i, tb in enumerate(tbs):
            blk = xc[toff[i]:toff[i] + tb]               # [tb, D]
            # [c, p, k, t] with d = (c*4+k)*128 + p
            m[f"xsT{i}"] = np.ascontiguousarray(
                blk.T.reshape(4, 4, P, tb).transpose(0, 2, 1, 3))
        in_maps.append(m)

    nc = get_nc(T, tbs)

    def assemble(res):
        out_flat = np.zeros((N, DIM), dtype=np.float32)
        for c in range(NCORES):
            toks = tok_of_slot[c * T:(c + 1) * T]
            valid = toks >= 0
            if valid.any():
                # yT [P, KB1, T] -> [T, D] with d = db*128 + p
                y = res[c]["yT"].transpose(2, 1, 0).reshape(T, DIM)
                out_flat[toks[valid]] = y[valid]
        return out_flat.reshape(bsz, seq, d)

    return nc, in_maps, assemble


def kernel(x, w_router, w_fc, w_proj):
    nc, in_maps, assemble = prepare(x, w_router, w_fc, w_proj)
    res = run_spmd(nc, in_maps)
    return assemble(res)



# revision 35
# speedup vs baseline: 67.2758x; 4.2292x over previous
"""MoE MLP (2 experts, top-1 routing) Trainium2 kernel.

Dispatch: tokens are sorted by routed expert and packed into 8
single-expert chunks.  The core split (c0 cores for expert 0, c1 = 8-c0
for expert 1) and the per-core token capacity T are chosen at runtime to
minimize T = max(ceil(n0/c0), ceil(n1/c1)) -- the per-core tensor work.
Top-1 routing sends each token to exactly one expert, so no cross-core
combine is needed; the host scatters rows back by token index.

Routing-weight folding: s(n) = top-prob of token n.  leaky_relu is
positively homogeneous and is squared, so
  s * square(leaky(x@W1.T)) @ W2.T == square(leaky((sqrt(s)*x)@W1.T)) @ W2.T
and sqrt(s) is folded into x on the host.

Device program (per core, operands host-packed, bf16 compute):
  hT = wfc @ xs        ([H,T], PSUM, 128x128 weight tiles, contraction D)
  aT = sq(lrelu(hT, 0.5))            (bf16, held in SBUF)
  yT = wpj @ aT        ([D,T], contraction H)  -> fp32 out

fp8 weight transport (the big lever): steady-state profiling showed the
kernel is DMA-BYTE-bound, not PE-bound -- the 67MB of bf16 weights
stream at only ~110-145GB/s effective while the PE needs them at
146GB/s, so every weight byte saved is time.  wpj ships as fp8 e3m4
(4 mantissa bits, x128 scale folded into the Square activation's input
scale); the proj matmul runs the e3m4 stationary against the bf16
moving aT at full rate (verified bit-exact upconversion on HW).  This
halves the wpj stream (33.5 -> 16.8MB) and measured -22% steady-state
(677 -> 525-548us/rep, drift-controlled interleaved A/B).  wfc must
stay bf16: its error doubles through the square (e3m4 there measures
~2.8e-2 > the 2e-2 gate).  End-to-end rel err 1.437e-2 (gate 2e-2).

DMA layouts:
  xsT[k]  [4, P, 4, tb_k]       4 lines of ~2KB per partition
  wfcT    [P, NPAN1, KB1, 256]  fc weights bf16, 8KB/partition per panel
  wpjT    [P, NPAN2, KB2, 256]  proj weights e3m4, 16KB/partition/panel
  yT      [P, KB1, T] fp32
Schedule: ~24 junk warmup matmuls un-gate the PE clock (HAM) while the
first DMAs land; wfc panel 0 arrives in 4 kb-chunks so the first matmul
group starts as soon as xs block 0 + 256KB of weights are in; wpj panels
0-2 prefetch in quarters interleaved with phase-1 wfc loads; the last
output group is split 194+65 cols so only a 65-col copy+DMA drains after
the final matmul.  PE floor (260-col streams at 112ns measured clean):
~462us; graded single-shot baseline was 484.5us in bf16.
"""

from collections import deque

import numpy as np
import ml_dtypes

P = 128
DIM = 2048
HID = 8192
NEXP = 2
NCORES = 8
NTOK = 4096
KB1 = DIM // P           # 16  fc contraction blocks
KB2 = HID // P           # 64  proj contraction blocks
HPAN = 2                 # h-blocks per fc weight panel
DPAN = 2                 # d-blocks per proj weight panel
NPAN1 = KB2 // HPAN      # 32
NPAN2 = KB1 // DPAN      # 8
# wpj ships as fp8 e3m4 (4 mantissa bits): the proj matmul runs the fp8
# stationary against the bf16 moving aT at full rate (verified exact on
# HW), halving the 33.5MB wpj DMA stream.  rel err 1.43e-2 measured on
# the real data (gate 2e-2); wfc must stay bf16 (its error doubles
# through the square).  The 1/WSCALE is folded into the Square scale.
WSCALE = 128.0           # |wpj|*128 < 7.7 << 15.5 (e3m4 max)

# A/B knobs (get_nc cache key includes them; prepare() packs to match)
FP8_WPJ = True           # ship wpj as e3m4 (else bf16)
XS_CHUNKS = 1            # dma_starts per xs tile
WPJ_BUFS = 3             # wpj pool depth
NPRE = 3                 # wpj panels prefetched during phase 1
TAIL_SPLIT = True        # split last output group 194+65
WFC_BUFS = 3             # wfc pool depth

_NC_CACHE = {}
_RUN_CACHE = {}
_W_CACHE = {}


# --------------------------------------------------------------------------
# device program
# --------------------------------------------------------------------------
def _cfg():
    return (FP8_WPJ, XS_CHUNKS, WPJ_BUFS, NPRE, TAIL_SPLIT, WFC_BUFS)


def _build_nc(T, tbs, reps=1):
    import concourse.mybir as mybir
    import concourse.tile as tile
    from concourse import bacc

    dt = mybir.dt
    wpj_dt = dt.float8e3 if FP8_WPJ else dt.bfloat16
    nc = bacc.Bacc(None, target_bir_lowering=False)
    # chunk-major layout: per partition, 4 DMA lines of ~2KB.  Line size
    # drives per-DMA-engine throughput (measured 13GB/s at 520B lines vs
    # 25GB/s at 2KB, x16 engines), and xs gates the first matmul group.
    xsT = [nc.dram_tensor(f"xsT{i}", [4, P, 4, tb], dt.bfloat16,
                          kind="ExternalInput").rearrange(
                              "c p k t -> p c k t")
           for i, tb in enumerate(tbs)]
    wfcT = nc.dram_tensor("wfcT", [P, NPAN1, KB1, HPAN * P], dt.bfloat16,
                          kind="ExternalInput")
    wpjT = nc.dram_tensor("wpjT", [P, NPAN2, KB2, DPAN * P], wpj_dt,
                          kind="ExternalInput")
    yT = nc.dram_tensor("yT", [P, KB1, T], dt.float32, kind="ExternalOutput")

    assert sum(tbs) == T and all(tb <= 512 for tb in tbs)
    toff = [sum(tbs[:i]) for i in range(len(tbs))]
    # phase-1 panel index -> list of (wpj_panel, quarter) prefetches
    wpj_pre = {}
    npre = min(NPRE, NPAN2)
    for i in range(npre * 4):
        wpj_pre.setdefault(8 + 2 * i, []).append((i // 4, i % 4))
    qkb = KB2 // 4           # kb-blocks per prefetch quarter

    with tile.TileContext(nc) as tc:
        with tc.tile_pool(name="xs", bufs=1) as xs_pool, \
             tc.tile_pool(name="wfc", bufs=WFC_BUFS) as wfc_pool, \
             tc.tile_pool(name="wpj", bufs=WPJ_BUFS) as wpj_pool, \
             tc.tile_pool(name="a", bufs=1) as a_pool, \
             tc.tile_pool(name="g", bufs=3) as g_pool, \
             tc.tile_pool(name="ps", bufs=8, space="PSUM") as ps_pool, \
             tc.tile_pool(name="ot", bufs=3) as out_pool:

            def load_wfc(pan, chunks=1):
                t = wfc_pool.tile([P, KB1, HPAN * P], dt.bfloat16,
                                  name="wfc_sb", tag="wfc")
                ck = KB1 // chunks
                for c in range(chunks):
                    nc.sync.dma_start(t[:, c * ck:(c + 1) * ck, :],
                                      wfcT[:, pan, c * ck:(c + 1) * ck, :])
                return t

            # HAM warmup: junk matmuls keep the PE busy while the first
            # DMAs land, so real matmuls start at the 2.4GHz clock and
            # the HAM never sees an idle window before they begin.
            wu = xs_pool.tile([P, P + tbs[0]], dt.bfloat16,
                              name="wu", tag="wu")
            nc.vector.memset(wu, 0.0)
            ps_w = ps_pool.tile([P, tbs[0]], dt.float32, tag="ps")
            for _ in range(24):
                nc.tensor.matmul(ps_w, wu[:, :P], wu[:, P:],
                                 start=True, stop=True)

            for _rep in range(reps):
                _body(nc, tile, mybir, dt, T, tbs, toff, wpj_pre, qkb,
                      xsT, wfcT, wpjT, yT, load_wfc,
                      xs_pool, wfc_pool, wpj_pool, a_pool, g_pool,
                      ps_pool, out_pool)
    nc.compile()
    return nc


def _body(nc, tile, mybir, dt, T, tbs, toff, wpj_pre, qkb,
          xsT, wfcT, wpjT, yT, load_wfc,
          xs_pool, wfc_pool, wpj_pool, a_pool, g_pool, ps_pool, out_pool):
    wpj_dt = dt.float8e3 if FP8_WPJ else dt.bfloat16
    sq_scale = 0.5 / float(np.sqrt(WSCALE)) if FP8_WPJ else 0.5
    # startup order: xs block 0 first, then wfc panel 0 in
    # kb-chunks -- the first matmul group only needs xs0 plus the
    # first chunk, so it starts ~5us earlier than whole-panel DMA
    xs_sb = []
    for i, tb in enumerate(tbs):
        # distinct tags: both token blocks stay live all of phase 1.
        # 4 dma_starts (one per c-chunk) pull on 4 queues concurrently,
        # cutting the xs0 arrival latency that gates the first matmul.
        t = xs_pool.tile([P, 4, 4, tb], dt.bfloat16,
                         name=f"xs{i}", tag=f"xs{i}")
        if XS_CHUNKS == 1:
            nc.sync.dma_start(t, xsT[i])
        else:
            for c in range(XS_CHUNKS):
                cs = 4 // XS_CHUNKS
                nc.sync.dma_start(t[:, c * cs:(c + 1) * cs],
                                  xsT[i][:, c * cs:(c + 1) * cs])
        xs_sb.append(t)
        if i == 0:
            wfc_q = deque([load_wfc(0, chunks=4)])
    for pan in range(1, min(WFC_BUFS, NPAN1)):
        wfc_q.append(load_wfc(pan))

    aT = a_pool.tile([P, KB2, T], dt.bfloat16, tag="aT")
    wpj_tiles = {}

    # ---- phase 1: hT = wfc @ xs; aT = sq(lrelu(hT, 0.5)) ----
    for pan in range(NPAN1):
        wfc_sb = wfc_q.popleft()
        if pan + WFC_BUFS < NPAN1:
            wfc_q.append(load_wfc(pan + WFC_BUFS))
        for wp, q in wpj_pre.get(pan, []):
            if wp not in wpj_tiles:
                wpj_tiles[wp] = wpj_pool.tile(
                    [P, KB2, DPAN * P], wpj_dt,
                    name=f"wpj_sb{wp}", tag="wpj")
            nc.sync.dma_start(
                wpj_tiles[wp][:, q * qkb:(q + 1) * qkb, :],
                wpjT[:, wp, q * qkb:(q + 1) * qkb, :])
        # panel 0: ti-outer so the first groups only need xs
        # block 0 (xs1's DMA is still in flight at that point)
        if pan == 0:
            groups = [(hb, ti) for ti in range(len(tbs))
                      for hb in range(HPAN)]
        else:
            groups = [(hb, ti) for hb in range(HPAN)
                      for ti in range(len(tbs))]
        for hb, ti in groups:
            tb = tbs[ti]
            t0 = toff[ti]
            ps = ps_pool.tile([P, tb], dt.float32, tag="ps")
            for kb in range(KB1):
                nc.tensor.matmul(
                    ps,
                    wfc_sb[:, kb, hb * P:(hb + 1) * P],
                    xs_sb[ti][:, kb // 4, kb % 4, :],
                    start=(kb == 0), stop=(kb == KB1 - 1))
            # sq(lrelu(h,.5)) == Square(0.5*(h + relu(h)))
            # (ActivationFunctionType.Lrelu ignores alpha on HW)
            r = g_pool.tile([P, tb], dt.float32, tag="r")
            nc.scalar.activation(
                r, ps, mybir.ActivationFunctionType.Relu)
            s = g_pool.tile([P, tb], dt.float32, tag="s")
            nc.vector.tensor_add(out=s, in0=ps, in1=r)
            # a' = a/WSCALE via Square's input scale: (c*x)^2, c=0.5/sqrt
            nc.scalar.activation(
                aT[:, pan * HPAN + hb, t0:t0 + tb],
                s, mybir.ActivationFunctionType.Square,
                scale=sq_scale)

    # ---- phase 2: yT = wpj @ aT ----
    for pan in range(NPAN2):
        if pan in wpj_tiles:
            wpj_sb = wpj_tiles.pop(pan)
        else:
            wpj_sb = wpj_pool.tile([P, KB2, DPAN * P], wpj_dt,
                                   tag="wpj")
            nc.sync.dma_start(wpj_sb, wpjT[:, pan])
        for db in range(DPAN):
            for ti, tb in enumerate(tbs):
                t0 = toff[ti]
                # drain tail: the very last group is split column-wise
                # into (tb-64, 64) PSUM groups so only a 64-col copy+DMA
                # is exposed after the final matmul.
                last = (TAIL_SPLIT and pan == NPAN2 - 1 and db == DPAN - 1
                        and ti == len(tbs) - 1 and tb > 96)
                splits = ((0, tb - 64), (tb - 64, tb)) if last \
                    else ((0, tb),)
                for lo, hi in splits:
                    ps = ps_pool.tile([P, hi - lo], dt.float32, tag="ps")
                    for kb in range(KB2):
                        nc.tensor.matmul(
                            ps,
                            wpj_sb[:, kb, db * P:(db + 1) * P],
                            aT[:, kb, t0 + lo:t0 + hi],
                            start=(kb == 0), stop=(kb == KB2 - 1))
                    ot = out_pool.tile([P, hi - lo], dt.float32, tag="o")
                    nc.vector.tensor_copy(ot, ps)
                    nc.sync.dma_start(
                        yT[:, pan * DPAN + db, t0 + lo:t0 + hi], ot)


def get_nc(T, tbs, reps=1):
    key = (T, tbs, reps, _cfg())
    if key not in _NC_CACHE:
        _NC_CACHE[key] = _build_nc(T, tbs, reps)
    return _NC_CACHE[key]


# --------------------------------------------------------------------------
# runner: build the sharded jit once per nc, reuse across calls
# --------------------------------------------------------------------------
def get_runner(nc, n_cores=NCORES):
    """Returns (fn, in_names, out_names, out_shapes).  fn takes
    [n_cores*dim0, ...] concatenated inputs + zero output buffers and
    returns concatenated outputs (mirrors bass2jax.run_bass_via_pjrt,
    but the jitted callable is cached so repeat calls don't recompile)."""
    key = id(nc)
    if key in _RUN_CACHE:
        return _RUN_CACHE[key]

    import jax
    import concourse.mybir as mybir
    from concourse.bass2jax import (
        _bass_exec_p, install_neuronx_cc_hook, partition_id_tensor)
    from jax.sharding import Mesh, PartitionSpec
    try:
        from jax.experimental.shard_map import shard_map
    except ImportError:
        from jax.shard_map import shard_map

    install_neuronx_cc_hook()

    part_name = (nc.partition_id_tensor.name
                 if nc.partition_id_tensor else None)
    in_names, out_names, out_avals = [], [], []
    for alloc in nc.m.functions[0].allocations:
        if not isinstance(alloc, mybir.MemoryLocationSet):
            continue
        name = alloc.memorylocations[0].name
        if alloc.kind == "ExternalInput":
            if name != part_name:
                in_names.append(name)
        elif alloc.kind == "ExternalOutput":
            out_names.append(name)
            out_avals.append(jax.core.ShapedArray(
                tuple(alloc.tensor_shape), mybir.dt.np(alloc.dtype)))
    n_params = len(in_names)
    n_outs = len(out_names)
    all_names = in_names + out_names
    if part_name is not None:
        all_names = all_names + [part_name]
    donate = tuple(range(n_params, n_params + n_outs))

    def _body(*args):
        operands = list(args)
        if part_name is not None:
            operands.append(partition_id_tensor())
        outs = _bass_exec_p.bind(
            *operands,
            out_avals=tuple(out_avals),
            in_names=tuple(all_names),
            out_names=tuple(out_names),
            lowering_input_output_aliases=(),
            sim_require_finite=True,
            sim_require_nnan=True,
            nc=nc,
        )
        return tuple(outs)

    devices = jax.devices()[:n_cores]
    mesh = Mesh(np.asarray(devices), ("core",))
    in_specs = (PartitionSpec("core"),) * (n_params + n_outs)
    out_specs = (PartitionSpec("core"),) * n_outs
    fn = jax.jit(
        shard_map(_body, mesh=mesh, in_specs=in_specs,
                  out_specs=out_specs, check_rep=False),
        donate_argnums=donate, keep_unused=True)
    out_shapes = [(tuple(a.shape), a.dtype) for a in out_avals]
    _RUN_CACHE[key] = (fn, in_names, out_names, out_shapes)
    return _RUN_CACHE[key]


def run_spmd(nc, in_maps, n_cores=NCORES):
    fn, in_names, out_names, out_shapes = get_runner(nc, n_cores)
    concat_in = [np.concatenate([m[n] for m in in_maps], axis=0)
                 for n in in_names]
    zeros = [np.zeros((n_cores * sh[0], *sh[1:]), dt)
             for sh, dt in out_shapes]
    outs = fn(*concat_in, *zeros)
    res = []
    for c in range(n_cores):
        res.append({
            name: np.asarray(outs[i]).reshape(n_cores, *out_shapes[i][0])[c]
            for i, name in enumerate(out_names)})
    return res


# --------------------------------------------------------------------------
# host dispatch
# --------------------------------------------------------------------------
def _route(x, w_router):
    """fp32 router matching reference: top = argmax(logits) (tie -> 0),
    s = top softmax prob = sigmoid(l_top - l_other)."""
    x_flat = np.asarray(x, dtype=np.float32).reshape(-1, x.shape[-1])
    L = x_flat @ np.asarray(w_router, dtype=np.float32).T
    top = (L[:, 1] > L[:, 0])
    dlt = np.abs(L[:, 1] - L[:, 0]).astype(np.float32)
    ptop = 1.0 / (1.0 + np.exp(-dlt))
    return x_flat, top, np.sqrt(ptop).astype(np.float32)


def _plan(n0, n1):
    """Core split minimizing per-core capacity T (the PE time scales
    linearly with T, so no rounding: every column costs ~0.85us/512)."""
    best = None
    for c0 in range(NCORES + 1):
        c1 = NCORES - c0
        if (n0 > 0 and c0 == 0) or (n1 > 0 and c1 == 0):
            continue
        T = max(-(-n0 // c0) if c0 else 0, -(-n1 // c1) if c1 else 0, 8)
        if best is None or T < best[0]:
            best = (T, c0)
    return best


def _pack_weights(w_fc, w_proj):
    """Panel-contiguous layouts (cached across calls; the harness reuses
    the same arrays).  wfcT bf16 [p,pan,kb,j] = w_fc[pan*256+j, kb*128+p];
    wpjT fp8e3m4 of w_proj*WSCALE, same index map."""
    key = (id(w_fc), id(w_proj), FP8_WPJ)
    hit = _W_CACHE.get(key)
    if hit is not None and hit[0] is w_fc and hit[1] is w_proj:
        return hit[2], hit[3]
    bf16 = ml_dtypes.bfloat16
    e3m4 = ml_dtypes.float8_e3m4
    wfcT, wpjT = [], []
    for e in range(NEXP):
        a = np.asarray(w_fc[e], np.float32).astype(bf16)
        wfcT.append(np.ascontiguousarray(
            a.reshape(NPAN1, HPAN * P, KB1, P).transpose(3, 0, 2, 1)))
        if FP8_WPJ:
            b = np.clip(np.asarray(w_proj[e], np.float32) * WSCALE,
                        -15.25, 15.25).astype(e3m4)
        else:
            b = np.asarray(w_proj[e], np.float32).astype(bf16)
        wpjT.append(np.ascontiguousarray(
            b.reshape(NPAN2, DPAN * P, KB2, P).transpose(3, 0, 2, 1)))
    _W_CACHE.clear()
    _W_CACHE[key] = (w_fc, w_proj, wfcT, wpjT)
    return wfcT, wpjT


def prepare(x, w_router, w_fc, w_proj):
    """Host dispatch: returns (nc, in_maps, assemble) so the same device
    program can be run via the cached jit path (kernel) or via
    run_bass_kernel_spmd with tracing (bench)."""
    bsz, seq, d = x.shape
    N = bsz * seq
    assert d == DIM and N == NTOK
    bf16 = ml_dtypes.bfloat16

    x_flat, top, sq = _route(x, w_router)
    n1 = int(top.sum())
    n0 = N - n1
    T, c0 = _plan(n0, n1)
    tbs = (T,) if T <= 512 else ((T // 2 + 3) // 4 * 4, 0)
    if len(tbs) == 2:
        tbs = (tbs[0], T - tbs[0])
    global _LAST_PLAN
    _LAST_PLAN = (T, tbs)

    wfcT, wpjT = _pack_weights(w_fc, w_proj)

    # sort tokens by expert into single-expert chunks of capacity T
    perm0 = np.nonzero(~top)[0]
    perm1 = np.nonzero(top)[0]
    xs_all = np.zeros((NCORES * T, DIM), dtype=np.float32)
    tok_of_slot = np.full(NCORES * T, -1, dtype=np.int64)
    xs_scaled = x_flat * sq[:, None]
    xs_all[:n0] = xs_scaled[perm0]
    tok_of_slot[:n0] = perm0
    off1 = c0 * T
    xs_all[off1:off1 + n1] = xs_scaled[perm1]
    tok_of_slot[off1:off1 + n1] = perm1

    toff = [sum(tbs[:i]) for i in range(len(tbs))]
    in_maps = []
    for c in range(NCORES):
        e = 0 if c < c0 else 1
        xc = xs_all[c * T:(c + 1) * T].astype(bf16)      # [T, D]
        m = {"wfcT": wfcT[e], "wpjT": wpjT[e]}
        for i, tb in enumerate(tbs):
            blk = xc[toff[i]:toff[i] + tb]               # [tb, D]
            # [c, p, k, t] with d = (c*4+k)*128 + p
            m[f"xsT{i}"] = np.ascontiguousarray(
                blk.T.reshape(4, 4, P, tb).transpose(0, 2, 1, 3))
        in_maps.append(m)

    nc = get_nc(T, tbs)

    def assemble(res):
        out_flat = np.zeros((N, DIM), dtype=np.float32)
        for c in range(NCORES):
            toks = tok_of_slot[c * T:(c + 1) * T]
            valid = toks >= 0
            if valid.any():
                # yT [P, KB1, T] -> [T, D] with d = db*128 + p
                y = res[c]["yT"].transpose(2, 1, 0).reshape(T, DIM)
                out_flat[toks[valid]] = y[valid]
        return out_flat.reshape(bsz, seq, d)

    return nc, in_maps, assemble


def kernel(x, w_router, w_fc, w_proj):
    nc, in_maps, assemble = prepare(x, w_router, w_fc, w_proj)
    res = run_spmd(nc, in_maps)
    return assemble(res)

